# revision 2
# baseline (speedup 1.0000x reference)
"""Trainium2 Bass kernel for local cost-volume correlation (FlowNet-style).

Problem: in1, in2 [B=8, C=256, H=96, W=128] fp32; out [B, 81, H, W] where
out[b, dy*9+dx, h, w] = mean_c in1[b,c,h,w] * in2[b,c,h+dy-4,w+dx-4] (zero pad).

Sharding: data-parallel over batch, one image per NeuronCore (8 cores).

Per-core algorithm (two output rows h per iteration):
  - PE computes Gram bands with 4 column-group (tile_position) matmuls per
    row, M=32 each: psum[w2, slot, u_local] = sum_c in1[c,h,w2] *
    in2pad[c,row(slot),u] where group j streams the u-window [32j, 32j+40)
    -- a free mod-32 shear.  in1 row is the stationary operand (fp16), the
    9 zero-padded in2 rows (rolling slot buffer) are the moving operand.
  - The needed values sit on diagonals u_local = (w2%32) + dxi.  Extraction
    refines the shear in stages: GPSIMD indirect_copy #1 (per-16-partition
    block offsets) -> mod-16; DVE stream_shuffle regroups partitions so each
    16-block holds a single (w2%16)//8 parity; indirect_copy #2 (per-block
    offsets, with the dy slot rotation folded into per-h index tables) ->
    mod-8.  One DVE masked multiply (mask[p, jj] = 1/C iff jj == p%8) +
    segmented reduce then extracts the 81 values per pixel exactly.
  - PE transpose whose "identity" is the inverse partition permutation
    assembles [81, h, w] output directly in dy-major channel order.
"""

import threading

import numpy as np

B, C, H, W = 8, 256, 96, 128
ND = 9            # displacement range per axis
NCH = ND * ND     # 81 output channels
CK = 2            # C // 128 contraction chunks
P = 128
NSLOT = 10        # rolling in2 row slots (2-row batching needs h-4..h+5)
SROW = 140        # padded in2 row width (>= 32*3 + 40)
UW = 40           # per-column-group u window
SW = NSLOT * UW   # 400, gram band row per h (after mod-32 shear)
PSROW = 512       # psum row pitch (bank-sized) for the 2-row tile
JW = 24           # gather1 width per slot (16 block + 8 disp)
G1ROW = ND * JW   # 216, gather1 output per row
NIDX = 2 * G1ROW  # 432 (two rows), multiple of 16
J2W = 16          # gather2 width per slot (8 block + 8 disp)
G2ROW = ND * J2W  # 144, gather2 output per row
NIDX2 = 2 * G2ROW  # 288
MW = 8            # mask window width after mod-8 shear

# stream_shuffle mask: group same (t//8) parity within each 32-quadrant
SHUF = list(range(0, 8)) + list(range(16, 24)) + list(range(8, 16)) + list(range(24, 32))

_cache = {}
_lock = threading.Lock()


def _wrap_idx(flat):
    """flat [8, n] per-core index lists -> wrapped [128, n//16] tensor."""
    n = flat.shape[1]
    out = np.zeros((P, n // 16), dtype=np.uint16)
    for q in range(8):
        for i in range(n):
            out[16 * q + (i % 16), i // 16] = flat[q, i]
    return out


def _host_tables():
    # gather1 (2 rows): slot selection + dy rotation folded in.  5 tables
    # indexed by hm = h % 10 (h even): row `row`, dy -> physical slot
    # (h + row + dy - 4) % 10; group q gathers
    # S[p, row*SW + slot_in*UW + 16*(q%2) + j], j in [0,24), dy-major output.
    tabs1 = []
    for hm in range(0, NSLOT, 2):
        flat1 = np.zeros((8, NIDX), dtype=np.uint16)
        for q in range(8):
            for row in range(2):
                for dy in range(ND):
                    slot_in = (hm + row + dy - 4) % NSLOT
                    for j in range(JW):
                        flat1[q, row * G1ROW + dy * JW + j] = (
                            row * SW + slot_in * UW + 16 * (q % 2) + j
                        )
        tabs1.append(_wrap_idx(flat1))
    gidx = np.stack(tabs1, axis=1).reshape(P, 5 * (NIDX // 16))

    # gather2 (after shuffle), static: group q gathers
    # qs[p, row*G1ROW + dy*JW + 8*(q%2) + j2]
    flat2 = np.zeros((8, NIDX2), dtype=np.uint16)
    for q in range(8):
        for row in range(2):
            for dy in range(ND):
                for j2 in range(J2W):
                    flat2[q, row * G2ROW + dy * J2W + j2] = (
                        row * G1ROW + dy * JW + 8 * (q % 2) + j2
                    )
    gidx2 = _wrap_idx(flat2)

    mask = np.zeros((P, MW), dtype=np.float16)
    for p in range(P):
        mask[p, p % 8] = 1.0 / C
    # inverse shuffle permutation matrix: perm[p_new, old(p_new)] = 1
    perm = np.zeros((P, P), dtype=np.float16)
    for s in range(4):
        for i in range(32):
            perm[32 * s + i, 32 * s + SHUF[i]] = 1.0
    return gidx, gidx2, mask, perm


def _build_nc():
    from contextlib import ExitStack

    import concourse.bass as bass
    import concourse.mybir as mybir
    import concourse.tile as tile
    from concourse import bacc

    f32 = mybir.dt.float32
    f16 = mybir.dt.float16
    u16 = mybir.dt.uint16

    nc = bacc.Bacc("TRN2", target_bir_lowering=False, debug=False)
    in1 = nc.declare_dram_parameter("in1", [C, H, W], f32, isOutput=False)
    in2 = nc.declare_dram_parameter("in2", [C, H, W], f32, isOutput=False)
    gidx = nc.declare_dram_parameter(
        "gidx", [P, 5 * (NIDX // 16)], u16, isOutput=False
    )
    gidx2 = nc.declare_dram_parameter("gidx2", [P, NIDX2 // 16], u16, isOutput=False)
    maskt = nc.declare_dram_parameter("maskt", [P, MW], f16, isOutput=False)
    permt = nc.declare_dram_parameter("permt", [P, P], f16, isOutput=False)
    out_t = nc.declare_dram_parameter("out", [NCH, H, W], f32, isOutput=True)

    in1r = in1[:].rearrange("(k p) h w -> p k h w", p=P)
    in2r = in2[:].rearrange("(k p) h w -> p k h w", p=P)

    with ExitStack() as ctx:
        tc = ctx.enter_context(tile.TileContext(nc))
        const = ctx.enter_context(tc.tile_pool(name="const", bufs=1))
        persist = ctx.enter_context(tc.tile_pool(name="persist", bufs=1))
        inp = ctx.enter_context(tc.tile_pool(name="inp", bufs=4))
        wrp = ctx.enter_context(tc.tile_pool(name="wrp", bufs=3))
        sp = ctx.enter_context(tc.tile_pool(name="sp", bufs=3))
        qp = ctx.enter_context(tc.tile_pool(name="qp", bufs=3))
        qsp = ctx.enter_context(tc.tile_pool(name="qsp", bufs=3))
        q8p = ctx.enter_context(tc.tile_pool(name="q8p", bufs=3))
        pp = ctx.enter_context(tc.tile_pool(name="pp", bufs=3))
        op = ctx.enter_context(tc.tile_pool(name="op", bufs=3))
        gram = ctx.enter_context(tc.tile_pool(name="gram", bufs=3, space="PSUM"))
        ptp = ctx.enter_context(tc.tile_pool(name="ptp", bufs=2, space="PSUM"))

        gidx1_tabs = []
        for it in range(5):
            g1 = const.tile([P, NIDX // 16], u16, name=f"g1_{it}")
            nc.sync.dma_start(
                out=g1[:],
                in_=gidx[:, it * (NIDX // 16) : (it + 1) * (NIDX // 16)],
            )
            gidx1_tabs.append(g1)
        gidx2_s = const.tile([P, NIDX2 // 16], u16)
        nc.sync.dma_start(out=gidx2_s[:], in_=gidx2[:])
        mask_s = const.tile([P, MW], f16)
        nc.sync.dma_start(out=mask_s[:], in_=maskt[:])
        perm_s = const.tile([P, P], f16)
        nc.sync.dma_start(out=perm_s[:], in_=permt[:])

        # rolling fp16 padded in2 rows: [p, k, slot, u]
        Rr = persist.tile([P, CK, NSLOT, SROW], f16)
        nc.vector.memset(Rr[:], 0.0)
        # output accumulator [channel, h, w]
        Tfull = persist.tile([NCH, H, W], f32)

        def load_cast_in2_pair(r):
            # loads rows r, r+1 (both must be < H)
            rin = inp.tile([P, CK, 2, W], f32, tag="rin")
            nc.sync.dma_start(out=rin[:], in_=in2r[:, :, r : r + 2, :])
            s = r % NSLOT
            if s + 1 <= NSLOT - 1:
                nc.gpsimd.tensor_copy(
                    out=Rr[:, :, s : s + 2, 4 : 4 + W], in_=rin[:]
                )
            else:  # slot wrap: 8 then 0
                nc.gpsimd.tensor_copy(
                    out=Rr[:, :, s, 4 : 4 + W], in_=rin[:, :, 0, :]
                )
                nc.gpsimd.tensor_copy(
                    out=Rr[:, :, 0, 4 : 4 + W], in_=rin[:, :, 1, :]
                )

        for r in range(0, 4, 2):
            load_cast_in2_pair(r)

        for h in range(0, H, 2):
            r = h + 4
            if r + 1 < H:
                load_cast_in2_pair(r)
            else:  # h = 92 or 94: rows r, r+1 >= 96 -> zero the slots
                nc.vector.memset(Rr[:, :, r % NSLOT, :], 0.0)
                nc.vector.memset(Rr[:, :, (r + 1) % NSLOT, :], 0.0)

            win = inp.tile([P, CK, 2, W], f32, tag="win")
            nc.sync.dma_start(out=win[:], in_=in1r[:, :, h : h + 2, :])
            wr = wrp.tile([P, CK, 2, W], f16)
            nc.scalar.copy(out=wr[:, 0], in_=win[:, 0])
            nc.vector.tensor_copy(out=wr[:, 1], in_=win[:, 1])

            # col-group matmuls: group j computes w2 in [32j, 32j+32) against
            # u in [32j, 32j+40) -> psum[32j:32j+32, row*PSROW + slot*40 + ...]
            ps = gram.tile([P, 2 * PSROW], f32)
            S = sp.tile([P, 2, SW], f16)
            for row in range(2):
                for k in range(CK):
                    for j in range(4):
                        nc.tensor.matmul(
                            out=ps[32 * j : 32 * j + 32, row * PSROW : row * PSROW + SW],
                            lhsT=wr[:, k, row, 32 * j : 32 * j + 32],
                            rhs=Rr[:, k, :, 32 * j : 32 * j + UW],
                            start=(k == 0),
                            stop=(k == CK - 1),
                            tile_position=(0, 32 * j),
                        )
                nc.scalar.copy(
                    out=S[:, row], in_=ps[:, row * PSROW : row * PSROW + SW]
                )

            q16 = qp.tile([P, NIDX], f16)
            nc.gpsimd.indirect_copy(
                out=q16[:], data=S[:].rearrange("p a b -> p (a b)"),
                idxs=gidx1_tabs[(h % NSLOT) // 2][:],
                i_know_ap_gather_is_preferred=True,
            )

            qs = qsp.tile([P, NIDX], f16)
            nc.vector.stream_shuffle(out=qs[:], in_=q16[:], mask=SHUF)

            q8 = q8p.tile([P, NIDX2], f16)
            nc.gpsimd.indirect_copy(
                out=q8[:], data=qs[:], idxs=gidx2_s[:],
                i_know_ap_gather_is_preferred=True,
            )

            # masked multiply:
            # prod[p, row, dy, dxi, jj] = q8[p, row*144 + dy*16 + dxi + jj] * mask[p, jj]
            prod = pp.tile([P, 2 * NCH * MW], f16)
            q8a = q8[:]
            # (row, dy) merged: row stride 144 = 9 * J2W, so one uniform dim
            in0 = bass.AP(
                tensor=q8a.tensor,
                offset=q8a.offset,
                ap=[q8a.ap[0], [J2W, 2 * ND], [1, ND], [1, MW]],
            )
            in1b = (
                mask_s[:]
                .unsqueeze(1)
                .unsqueeze(1)
                .to_broadcast([P, 2 * ND, ND, MW])
            )
            nc.vector.tensor_mul(
                prod[:].rearrange("p (a b c) -> p a b c", b=ND, c=MW),
                in0,
                in1b,
            )

            # pairwise add tree (tensor_tensor runs at 2x, tensor_reduce at 1x)
            pr4 = prod[:].rearrange("p (a c) -> p a c", c=MW)
            t1 = op.tile([P, 2 * NCH, 4], f16, name="t1", tag="t1")
            nc.vector.tensor_add(t1[:], pr4[:, :, 0:4], pr4[:, :, 4:8])
            t2 = op.tile([P, 2 * NCH, 2], f16, name="t2", tag="t2")
            nc.vector.tensor_add(t2[:], t1[:, :, 0:2], t1[:, :, 2:4])
            O = op.tile([P, 2 * NCH], f16)
            nc.vector.tensor_add(O[:], t2[:, :, 0], t2[:, :, 1])

            # transpose via PE; rhs = inverse shuffle permutation, so columns
            # land at the true pixel positions.
            for row in range(2):
                pt = ptp.tile([NCH, P], f16, name=f"pt{row}", tag="pt")
                nc.tensor.transpose(
                    out=pt[:], in_=O[:, row * NCH : (row + 1) * NCH],
                    identity=perm_s[:],
                )
                nc.scalar.copy(out=Tfull[:, h + row, :], in_=pt[:])

            if h % 8 == 6:
                nc.sync.dma_start(
                    out=out_t[:, h - 6 : h + 2, :], in_=Tfull[:, h - 6 : h + 2, :]
                )

    nc.finalize()
    return nc


def _get_nc():
    with _lock:
        if "nc" not in _cache:
            _cache["nc"] = _build_nc()
        return _cache["nc"]


def _in_maps(in1: np.ndarray, in2: np.ndarray):
    gidx, gidx2, mask, perm = _host_tables()
    in1 = np.ascontiguousarray(in1, dtype=np.float32)
    in2 = np.ascontiguousarray(in2, dtype=np.float32)
    return [
        {
            "in1": in1[b],
            "in2": in2[b],
            "gidx": gidx,
            "gidx2": gidx2,
            "maskt": mask,
            "permt": perm,
        }
        for b in range(B)
    ]


def kernel(in1: np.ndarray, in2: np.ndarray) -> np.ndarray:
    from concourse.bass_utils import run_bass_kernel_spmd

    nc = _get_nc()
    in_maps = _in_maps(in1, in2)
    res = run_bass_kernel_spmd(nc, in_maps, core_ids=list(range(B)))
    out = np.stack([res.results[b]["out"] for b in range(B)], axis=0)
    return out



# revision 3
# speedup vs baseline: 3.1768x; 3.1768x over previous
"""Trainium2 Bass kernel for local cost-volume correlation (FlowNet-style).

Problem: in1, in2 [B=8, C=256, H=96, W=128] fp32; out [B, 81, H, W] where
out[b, dy*9+dx, h, w] = mean_c in1[b,c,h,w] * in2[b,c,h+dy-4,w+dx-4] (zero pad).

Sharding: data-parallel over batch, one image per NeuronCore (8 cores).

Per-core pipeline (two output rows h per iteration):
  - in2 rows live in a 16-slot ring with the first 10 slots mirrored (+16), so
    any 9-slot dy-window is a CONTIGUOUS slice and all downstream index tables
    are static.  Rows are cast f32->f16 by ACT straight into ring+mirror; the
    ring is loaded 2 iterations ahead so the PE never waits.
  - PE computes Gram bands with 4 column-group (tile_position) matmuls per
    (row, k): psum[w2, dy*40+u_local] = sum_c in1[c,h,w2]*in2[c,h+dy-4,u],
    group j streaming the u-window [32j, 32j+40) of 9 slots (N=360).
  - ACT copies each row band psum->SBUF f16.  Extraction refines the mod-32
    diagonal shear in stages: gpsimd ap_gather #1 (8-elem chunks, per-parity
    block offsets) -> mod-16; DVE stream_shuffle regroups partitions; gather #2
    -> mod-8.  DVE masked multiplies (even/odd dxi split keeps APs 4B-aligned
    for 2x mode) + pairwise add tree select the exact value per (pixel, ch).
  - PE transpose (identity = inverse shuffle permutation) -> [81, w] channel-
    major; ACT stages to SBUF f32; one output DMA per iteration (no persistent
    accumulator -> no cross-iteration serialization).
"""

import threading

import numpy as np

B, C, H, W = 8, 256, 96, 128
ND = 9             # displacement range per axis
NCH = ND * ND      # 81 output channels
CK = 2             # C // 128 contraction chunks
P = 128
NSLOTP = 16        # physical ring slots
NMIR = 10          # slots 0..9 mirrored at +16
NSLOT = NSLOTP + NMIR  # 26
SROW = 140         # padded in2 row width (4 + 128 + 8)
UW = 40            # per-column-group u window
BAND = ND * UW     # 360, gram band row per h
PSROW = 512        # psum row pitch (bank-sized)
G1D = 8            # gather chunk width
G1N = 64           # gather1 num_idxs (54 real + pad)
G2N = 48           # gather2 num_idxs (36 real + pad)

# stream_shuffle mask: group same (t//8) parity within each 32-quadrant
SHUF = list(range(0, 8)) + list(range(16, 24)) + list(range(8, 16)) + list(range(24, 32))

_cache = {}
_lock = threading.Lock()


def _wrap_idx(flat, ncols):
    """flat [8, n] per-block index lists -> wrapped [128, ncols] tensor."""
    n = flat.shape[1]
    out = np.zeros((P, ncols), dtype=np.int16)
    for q in range(8):
        for i in range(n):
            out[16 * q + (i % 16), i // 16] = flat[q, i]
    return out


def _host_tables():
    # gather1: S viewed [128, 90, 8]; for (row, dy) gather 3 chunks starting
    # at (row*360 + dy*40 + 16*(q%2))/8; output q16 [128, 64, 8] -> mod-16.
    flat1 = np.zeros((8, G1N), dtype=np.int16)
    for q in range(8):
        for i in range(54):
            row, rem = divmod(i, 27)
            dy, c = divmod(rem, 3)
            flat1[q, i] = row * 45 + dy * 5 + 2 * (q % 2) + c
    g1 = _wrap_idx(flat1, G1N // 16)

    # gather2: qs viewed [128, 54, 8]; for (row, dy) gather 2 chunks starting
    # at (row*216 + dy*24 + 8*(q%2))/8; output q8 [128, 48, 8] -> mod-8.
    flat2 = np.zeros((8, G2N), dtype=np.int16)
    for q in range(8):
        for i in range(36):
            row, rem = divmod(i, 18)
            dy, c = divmod(rem, 2)
            flat2[q, i] = row * 27 + dy * 3 + (q % 2) + c
    g2 = _wrap_idx(flat2, G2N // 16)

    mask = np.zeros((P, 8), dtype=np.float16)
    for p in range(P):
        mask[p, p % 8] = 1.0 / C
    # inverse shuffle permutation matrix: perm[p_new, old(p_new)] = 1
    perm = np.zeros((P, P), dtype=np.float16)
    for s in range(4):
        for i in range(32):
            perm[32 * s + i, 32 * s + SHUF[i]] = 1.0
    return g1, g2, mask, perm


def _build_nc():
    from contextlib import ExitStack

    import concourse.bass as bass
    import concourse.mybir as mybir
    import concourse.tile as tile
    from concourse import bacc

    f32 = mybir.dt.float32
    f16 = mybir.dt.float16
    i16 = mybir.dt.int16

    nc = bacc.Bacc("TRN2", target_bir_lowering=False, debug=False)
    in1 = nc.declare_dram_parameter("in1", [C, H, W], f32, isOutput=False)
    in2 = nc.declare_dram_parameter("in2", [C, H, W], f32, isOutput=False)
    g1t = nc.declare_dram_parameter("g1t", [P, G1N // 16], i16, isOutput=False)
    g2t = nc.declare_dram_parameter("g2t", [P, G2N // 16], i16, isOutput=False)
    maskt = nc.declare_dram_parameter("maskt", [P, 8], f16, isOutput=False)
    permt = nc.declare_dram_parameter("permt", [P, P], f16, isOutput=False)
    out_t = nc.declare_dram_parameter("out", [NCH, H, W], f32, isOutput=True)

    in1r = in1[:].rearrange("(k p) h w -> p k h w", p=P)
    in2r = in2[:].rearrange("(k p) h w -> p k h w", p=P)

    with ExitStack() as ctx:
        tc = ctx.enter_context(tile.TileContext(nc))
        const = ctx.enter_context(tc.tile_pool(name="const", bufs=1))
        persist = ctx.enter_context(tc.tile_pool(name="persist", bufs=1))
        inp = ctx.enter_context(tc.tile_pool(name="inp", bufs=4))
        wrp = ctx.enter_context(tc.tile_pool(name="wrp", bufs=3))
        sp = ctx.enter_context(tc.tile_pool(name="sp", bufs=3))
        qp = ctx.enter_context(tc.tile_pool(name="qp", bufs=3))
        qsp = ctx.enter_context(tc.tile_pool(name="qsp", bufs=3))
        q8p = ctx.enter_context(tc.tile_pool(name="q8p", bufs=3))
        pp = ctx.enter_context(tc.tile_pool(name="pp", bufs=3))
        op = ctx.enter_context(tc.tile_pool(name="op", bufs=3))
        stg = ctx.enter_context(tc.tile_pool(name="stg", bufs=3))
        gram = ctx.enter_context(tc.tile_pool(name="gram", bufs=3, space="PSUM"))
        ptp = ctx.enter_context(tc.tile_pool(name="ptp", bufs=2, space="PSUM"))

        g1_s = const.tile([P, G1N // 16], i16)
        nc.sync.dma_start(out=g1_s[:], in_=g1t[:])
        g2_s = const.tile([P, G2N // 16], i16)
        nc.sync.dma_start(out=g2_s[:], in_=g2t[:])
        mask_s = const.tile([P, 8], f16)
        nc.sync.dma_start(out=mask_s[:], in_=maskt[:])
        perm_s = const.tile([P, P], f16)
        nc.sync.dma_start(out=perm_s[:], in_=permt[:])

        # in2 ring: [p, k, slot, u]; slot(r) = (r+4) % 16, mirrored at +16
        Rr = persist.tile([P, CK, NSLOT, SROW], f16)
        nc.vector.memset(Rr[:], 0.0)

        def cast_in2_pair(rin, s):
            # rin [P, CK, 2, W] f32 -> ring slots s, s+1 (cols 4:132), plus
            # mirror copies at s+16 when s <= 8 (s is always even here).
            if s <= 8:
                for k in range(CK):
                    src = bass.AP(
                        tensor=rin[:].tensor,
                        offset=rin[:].offset + k * (2 * W),
                        ap=[rin[:].ap[0], [0, 2], [W, 2], [1, W]],
                    )
                    dst = bass.AP(
                        tensor=Rr[:].tensor,
                        offset=Rr[:].offset + k * (NSLOT * SROW) + s * SROW + 4,
                        ap=[Rr[:].ap[0], [16 * SROW, 2], [SROW, 2], [1, W]],
                    )
                    nc.scalar.copy(out=dst, in_=src)
            else:
                nc.scalar.copy(
                    out=Rr[:, :, s : s + 2, 4 : 4 + W], in_=rin[:]
                )

        def load_in2_pair(r):
            rin = inp.tile([P, CK, 2, W], f32, tag="rin")
            nc.sync.dma_start(out=rin[:], in_=in2r[:, :, r : r + 2, :])
            cast_in2_pair(rin, (r + 4) % NSLOTP)

        for r in range(0, 8, 2):
            load_in2_pair(r)

        for h in range(0, H, 2):
            # prefetch in2 rows h+8, h+9 (2 iterations ahead)
            if h <= 86:
                load_in2_pair(h + 8)
            elif h in (88, 90):
                s = (h + 12) % NSLOTP
                for k in range(CK):
                    nc.vector.memset(Rr[:, k, s : s + 2, :], 0.0)
                    nc.vector.memset(Rr[:, k, s + 16 : s + 18, :], 0.0)

            win = inp.tile([P, CK, 2, W], f32, tag="win")
            nc.sync.dma_start(out=win[:], in_=in1r[:, :, h : h + 2, :])
            wr = wrp.tile([P, CK, 2, W], f16)
            nc.vector.tensor_copy(out=wr[:], in_=win[:])

            # col-group matmuls: group j computes w2 in [32j, 32j+32) against
            # u in [32j, 32j+40) over the 9-slot dy window (contiguous via
            # mirror) -> psum[32j:32j+32, row*PSROW + dy*40 + u_local]
            ps = gram.tile([P, 2 * PSROW], f32)
            S = sp.tile([P, 2, BAND], f16)
            for row in range(2):
                sr = (h + row) % NSLOTP
                for k in range(CK):
                    for j in range(4):
                        nc.tensor.matmul(
                            out=ps[32 * j : 32 * j + 32, row * PSROW : row * PSROW + BAND],
                            lhsT=wr[:, k, row, 32 * j : 32 * j + 32],
                            rhs=Rr[:, k, sr : sr + ND, 32 * j : 32 * j + UW],
                            start=(k == 0),
                            stop=(k == CK - 1),
                            tile_position=(0, 32 * j),
                        )
                nc.scalar.copy(
                    out=S[:, row], in_=ps[:, row * PSROW : row * PSROW + BAND]
                )

            # gather1 -> mod-16 (8-elem chunks; static table)
            q16 = qp.tile([P, G1N, G1D], f16)
            nc.gpsimd.ap_gather(
                out_ap=q16[:],
                in_ap=S[:].rearrange("p a b -> p (a b)").rearrange(
                    "p (n d) -> p n d", d=G1D
                ),
                idxs_ap=g1_s[:],
                channels=P,
                num_elems=2 * BAND // G1D,
                d=G1D,
                num_idxs=G1N,
            )

            qs = qsp.tile([P, 2 * ND * 24], f16)
            nc.vector.stream_shuffle(
                out=qs[:],
                in_=q16[:].rearrange("p a b -> p (a b)")[:, 0 : 2 * ND * 24],
                mask=SHUF,
            )

            # gather2 -> mod-8
            q8 = q8p.tile([P, G2N, G1D], f16)
            nc.gpsimd.ap_gather(
                out_ap=q8[:],
                in_ap=qs[:].rearrange("p (n d) -> p n d", d=G1D),
                idxs_ap=g2_s[:],
                channels=P,
                num_elems=2 * ND * 24 // G1D,
                d=G1D,
                num_idxs=G2N,
            )

            # masked select: value for (p, rd, dxi) at q8[p, rd*16 + dxi + p%8].
            # Split even/odd dxi so every inner run starts 4B-aligned (2x mode
            # on the even half); odd half runs 1x.
            q8a = q8[:].rearrange("p a b -> p (a b)")
            RD = 2 * ND  # 18 (row, dy) pairs
            in_e = bass.AP(
                tensor=q8a.tensor, offset=q8a.offset,
                ap=[q8a.ap[0], [16, RD], [2, 5], [1, 8]],
            )
            in_o = bass.AP(
                tensor=q8a.tensor, offset=q8a.offset + 1,
                ap=[q8a.ap[0], [16, RD], [2, 4], [1, 8]],
            )
            mb_e = mask_s[:].unsqueeze(1).unsqueeze(1).to_broadcast([P, RD, 5, 8])
            mb_o = mask_s[:].unsqueeze(1).unsqueeze(1).to_broadcast([P, RD, 4, 8])
            pe_ = pp.tile([P, RD, 5, 8], f16, name="pe", tag="pe")
            po_ = pp.tile([P, RD, 4, 8], f16, name="po", tag="po")
            nc.vector.tensor_mul(pe_[:], in_e, mb_e)
            nc.vector.tensor_mul(po_[:], in_o, mb_o)

            t1e = op.tile([P, RD, 5, 4], f16, name="t1e", tag="t1e")
            t1o = op.tile([P, RD, 4, 4], f16, name="t1o", tag="t1o")
            nc.vector.tensor_add(t1e[:], pe_[:, :, :, 0:4], pe_[:, :, :, 4:8])
            nc.vector.tensor_add(t1o[:], po_[:, :, :, 0:4], po_[:, :, :, 4:8])
            t2e = op.tile([P, RD, 5, 2], f16, name="t2e", tag="t2e")
            t2o = op.tile([P, RD, 4, 2], f16, name="t2o", tag="t2o")
            nc.vector.tensor_add(t2e[:], t1e[:, :, :, 0:2], t1e[:, :, :, 2:4])
            nc.vector.tensor_add(t2o[:], t1o[:, :, :, 0:2], t1o[:, :, :, 2:4])

            O = op.tile([P, RD * ND], f16, name="O", tag="O")
            Oa = O[:]
            out_e = bass.AP(
                tensor=Oa.tensor, offset=Oa.offset,
                ap=[Oa.ap[0], [ND, RD], [2, 5]],
            )
            out_o = bass.AP(
                tensor=Oa.tensor, offset=Oa.offset + 1,
                ap=[Oa.ap[0], [ND, RD], [2, 4]],
            )
            nc.vector.tensor_add(out_e, t2e[:, :, :, 0], t2e[:, :, :, 1])
            nc.vector.tensor_add(out_o, t2o[:, :, :, 0], t2o[:, :, :, 1])

            # transpose via PE; identity = inverse shuffle permutation, so
            # columns land at the true pixel positions.  Stage + DMA out.
            Ost = stg.tile([NCH, 2, W], f32)
            for row in range(2):
                pt = ptp.tile([P, P], f16, name=f"pt{row}", tag="pt")
                nc.tensor.transpose(
                    out=pt[0:NCH, :], in_=O[:, row * NCH : (row + 1) * NCH],
                    identity=perm_s[:],
                )
                nc.scalar.copy(out=Ost[:, row, :], in_=pt[0:NCH, :])
            nc.sync.dma_start(out=out_t[:, h : h + 2, :], in_=Ost[:])

    nc.finalize()
    return nc


def _get_nc():
    with _lock:
        if "nc" not in _cache:
            _cache["nc"] = _build_nc()
        return _cache["nc"]


def _in_maps(in1: np.ndarray, in2: np.ndarray):
    g1, g2, mask, perm = _host_tables()
    in1 = np.ascontiguousarray(in1, dtype=np.float32)
    in2 = np.ascontiguousarray(in2, dtype=np.float32)
    return [
        {
            "in1": in1[b],
            "in2": in2[b],
            "g1t": g1,
            "g2t": g2,
            "maskt": mask,
            "permt": perm,
        }
        for b in range(B)
    ]


def kernel(in1: np.ndarray, in2: np.ndarray) -> np.ndarray:
    from concourse.bass_utils import run_bass_kernel_spmd

    nc = _get_nc()
    in_maps = _in_maps(in1, in2)
    res = run_bass_kernel_spmd(nc, in_maps, core_ids=list(range(B)))
    out = np.stack([res.results[b]["out"] for b in range(B)], axis=0)
    return out


# revision 4
# speedup vs baseline: 4.2503x; 1.3379x over previous
"""Trainium2 Bass kernel for local cost-volume correlation (FlowNet-style).

Problem: in1, in2 [B=8, C=256, H=96, W=128] fp32; out [B, 81, H, W] where
out[b, dy*9+dx, h, w] = mean_c in1[b,c,h,w] * in2[b,c,h+dy-4,w+dx-4] (zero pad).

Sharding: data-parallel over batch, one image per NeuronCore (8 cores).

Per-core pipeline (FOUR output rows h per iteration; matmul/psum at 2-row
granularity, extraction batched over 4 rows to amortize per-op fixed costs):
  - in2 rows live in a 16-slot ring with the first 10 slots mirrored (+16), so
    any 9-slot dy-window is a CONTIGUOUS slice and all index tables static.
    Rows are cast f32->f16 by ACT straight into ring+mirror, loaded one
    iteration ahead.
  - PE computes Gram bands with 4 column-group (tile_position) matmuls per
    (row, k): psum[w2, dy*40+u_local] = sum_c in1[c,h,w2]*in2[c,h+dy-4,u],
    group j streaming u-window [32j, 32j+40) over the 9-slot dy window (N=360).
  - ACT copies two 2-row psum tiles into one 4-row S band (f16).  Extraction
    refines the mod-32 diagonal shear: gpsimd ap_gather #1 (8-elem chunks,
    per-parity block offsets) -> mod-16; DVE stream_shuffle regroups
    partitions; ap_gather #2 -> mod-8.  DVE masked multiplies (even dxi on q8,
    odd dxi on a +1-shifted copy, so every AP is 4B-aligned for 2x mode) +
    pairwise add tree select the exact value per (pixel, ch).
  - 4 PE transposes (identity = inverse shuffle permutation) accumulate into
    one PSUM tile -> [81, 4, w] channel-major; one ACT staging copy; one
    output DMA per iteration.
"""

import threading

import numpy as np

B, C, H, W = 8, 256, 96, 128
ND = 9             # displacement range per axis
NCH = ND * ND      # 81 output channels
CK = 2             # C // 128 contraction chunks
P = 128
RPI = 4            # rows per iteration
RD = RPI * ND      # 36 (row, dy) pairs per iteration
NSLOTP = 16        # physical ring slots
NMIR = 10          # slots 0..9 mirrored at +16
NSLOT = NSLOTP + NMIR  # 26
SROW = 140         # padded in2 row width (4 + 128 + 8)
UW = 40            # per-column-group u window
BAND = ND * UW     # 360, gram band per row
PSROW = 512        # psum row pitch (bank-sized)
G1D = 8            # gather chunk width
G1N = 112          # gather1 num_idxs (108 real + pad)
G2N = 80           # gather2 num_idxs (72 real + pad)

# stream_shuffle mask: group same (t//8) parity within each 32-quadrant
SHUF = list(range(0, 8)) + list(range(16, 24)) + list(range(8, 16)) + list(range(24, 32))

_cache = {}
_lock = threading.Lock()


def _wrap_idx(flat, ncols):
    n = flat.shape[1]
    out = np.zeros((P, ncols), dtype=np.int16)
    for q in range(8):
        for i in range(n):
            out[16 * q + (i % 16), i // 16] = flat[q, i]
    return out


def _host_tables():
    # gather1: S viewed [128, 180, 8]; for (row, dy) gather 3 chunks starting
    # at (row*360 + dy*40 + 16*(q%2))/8; output q16 [128, 112, 8] -> mod-16.
    flat1 = np.zeros((8, G1N), dtype=np.int16)
    for q in range(8):
        for i in range(RD * 3):
            row, rem = divmod(i, 27)
            dy, c = divmod(rem, 3)
            flat1[q, i] = row * 45 + dy * 5 + 2 * (q % 2) + c
    g1 = _wrap_idx(flat1, G1N // 16)

    # gather2: qs viewed [128, 108, 8]; for (row, dy) gather 2 chunks starting
    # at (row*216 + dy*24 + 8*(q%2))/8; output q8 [128, 80, 8] -> mod-8.
    flat2 = np.zeros((8, G2N), dtype=np.int16)
    for q in range(8):
        for i in range(RD * 2):
            row, rem = divmod(i, 18)
            dy, c = divmod(rem, 2)
            flat2[q, i] = row * 27 + dy * 3 + (q % 2) + c
    g2 = _wrap_idx(flat2, G2N // 16)

    # fully-materialized masks (no stride-0 dims -> DVE 2x mode)
    me = np.zeros((P, RD, 5, 8), dtype=np.float16)
    mo = np.zeros((P, RD, 4, 8), dtype=np.float16)
    for p in range(P):
        me[p, :, :, p % 8] = 1.0 / C
        mo[p, :, :, p % 8] = 1.0 / C
    # inverse shuffle permutation matrix: perm[p_new, old(p_new)] = 1
    perm = np.zeros((P, P), dtype=np.float16)
    for s in range(4):
        for i in range(32):
            perm[32 * s + i, 32 * s + SHUF[i]] = 1.0
    return g1, g2, me.reshape(P, -1), mo.reshape(P, -1), perm


def _build_nc():
    from contextlib import ExitStack

    import concourse.bass as bass
    import concourse.mybir as mybir
    import concourse.tile as tile
    from concourse import bacc

    f32 = mybir.dt.float32
    f16 = mybir.dt.float16
    i16 = mybir.dt.int16

    nc = bacc.Bacc("TRN2", target_bir_lowering=False, debug=False)
    in1 = nc.declare_dram_parameter("in1", [C, H, W], f32, isOutput=False)
    in2 = nc.declare_dram_parameter("in2", [C, H, W], f32, isOutput=False)
    g1t = nc.declare_dram_parameter("g1t", [P, G1N // 16], i16, isOutput=False)
    g2t = nc.declare_dram_parameter("g2t", [P, G2N // 16], i16, isOutput=False)
    met = nc.declare_dram_parameter("met", [P, RD * 40], f16, isOutput=False)
    mot = nc.declare_dram_parameter("mot", [P, RD * 32], f16, isOutput=False)
    permt = nc.declare_dram_parameter("permt", [P, P], f16, isOutput=False)
    out_t = nc.declare_dram_parameter("out", [NCH, H, W], f32, isOutput=True)

    in1r = in1[:].rearrange("(k p) h w -> p k h w", p=P)
    in2r = in2[:].rearrange("(k p) h w -> p k h w", p=P)

    with ExitStack() as ctx:
        tc = ctx.enter_context(tile.TileContext(nc))
        const = ctx.enter_context(tc.tile_pool(name="const", bufs=1))
        persist = ctx.enter_context(tc.tile_pool(name="persist", bufs=1))
        inp = ctx.enter_context(tc.tile_pool(name="inp", bufs=3))
        wrp = ctx.enter_context(tc.tile_pool(name="wrp", bufs=3))
        sp = ctx.enter_context(tc.tile_pool(name="sp", bufs=3))
        qp = ctx.enter_context(tc.tile_pool(name="qp", bufs=3))
        qsp = ctx.enter_context(tc.tile_pool(name="qsp", bufs=3))
        q8p = ctx.enter_context(tc.tile_pool(name="q8p", bufs=3))
        pp = ctx.enter_context(tc.tile_pool(name="pp", bufs=3))
        op = ctx.enter_context(tc.tile_pool(name="op", bufs=3))
        stg = ctx.enter_context(tc.tile_pool(name="stg", bufs=3))
        gram = ctx.enter_context(tc.tile_pool(name="gram", bufs=3, space="PSUM"))
        ptp = ctx.enter_context(tc.tile_pool(name="ptp", bufs=2, space="PSUM"))

        g1_s = const.tile([P, G1N // 16], i16)
        nc.sync.dma_start(out=g1_s[:], in_=g1t[:])
        g2_s = const.tile([P, G2N // 16], i16)
        nc.sync.dma_start(out=g2_s[:], in_=g2t[:])
        me_s = const.tile([P, RD * 40], f16)
        nc.sync.dma_start(out=me_s[:], in_=met[:])
        mo_s = const.tile([P, RD * 32], f16)
        nc.sync.dma_start(out=mo_s[:], in_=mot[:])
        perm_s = const.tile([P, P], f16)
        nc.sync.dma_start(out=perm_s[:], in_=permt[:])

        # in2 ring: [p, k, slot, u]; slot(r) = (r+4) % 16, mirrored at +16
        Rr = persist.tile([P, CK, NSLOT, SROW], f16)
        nc.vector.memset(Rr[:], 0.0)

        def cast_in2_quad(rin, s):
            # rin [P, CK, 4, W] f32 -> ring slots s..s+3 (cols 4:132), plus
            # mirror at +16 for slots <= 9.  s is a multiple of 4.
            nm = sum(1 for i in range(4) if s + i <= 9)  # 0, 2, or 4
            if nm == 4:
                for k in range(CK):
                    src = bass.AP(
                        tensor=rin[:].tensor,
                        offset=rin[:].offset + k * (4 * W),
                        ap=[rin[:].ap[0], [0, 2], [W, 4], [1, W]],
                    )
                    dst = bass.AP(
                        tensor=Rr[:].tensor,
                        offset=Rr[:].offset + k * (NSLOT * SROW) + s * SROW + 4,
                        ap=[Rr[:].ap[0], [16 * SROW, 2], [SROW, 4], [1, W]],
                    )
                    nc.scalar.copy(out=dst, in_=src)
            else:
                nc.scalar.copy(out=Rr[:, :, s : s + 4, 4 : 4 + W], in_=rin[:])
                if nm:
                    nc.scalar.copy(
                        out=Rr[:, :, s + 16 : s + 16 + nm, 4 : 4 + W],
                        in_=rin[:, :, 0:nm, :],
                    )

        def load_in2_quad(r):
            rin = inp.tile([P, CK, RPI, W], f32, tag="rin")
            nc.sync.dma_start(out=rin[:], in_=in2r[:, :, r : r + 4, :])
            cast_in2_quad(rin, (r + 4) % NSLOTP)

        for r in range(0, 8, 4):
            load_in2_quad(r)

        for h in range(0, H, RPI):
            # prefetch in2 rows h+8..h+11 (one iteration ahead)
            if h <= 84:
                load_in2_quad(h + 8)
            elif h == 88:
                s = (h + 12) % NSLOTP  # 4
                for k in range(CK):
                    nc.vector.memset(Rr[:, k, s : s + 4, :], 0.0)
                    nc.vector.memset(Rr[:, k, s + 16 : s + 20, :], 0.0)

            win = inp.tile([P, CK, RPI, W], f32, tag="win")
            nc.sync.dma_start(out=win[:], in_=in1r[:, :, h : h + RPI, :])
            wr = wrp.tile([P, CK, RPI, W], f16)
            nc.vector.tensor_copy(out=wr[:], in_=win[:])

            # col-group matmuls: group j computes w2 in [32j, 32j+32) against
            # u in [32j, 32j+40) over the 9-slot dy window (contiguous via
            # mirror) -> psum[32j:32j+32, (row%2)*PSROW + dy*40 + u_local]
            S = sp.tile([P, RPI, BAND], f16)
            for half in range(2):
                ps = gram.tile([P, 2 * PSROW], f32, tag="ps")
                for row2 in range(2):
                    row = 2 * half + row2
                    sr = (h + row) % NSLOTP
                    for k in range(CK):
                        for j in range(4):
                            nc.tensor.matmul(
                                out=ps[
                                    32 * j : 32 * j + 32,
                                    row2 * PSROW : row2 * PSROW + BAND,
                                ],
                                lhsT=wr[:, k, row, 32 * j : 32 * j + 32],
                                rhs=Rr[:, k, sr : sr + ND, 32 * j : 32 * j + UW],
                                start=(k == 0),
                                stop=(k == CK - 1),
                                tile_position=(0, 32 * j),
                            )
                # one copy per psum tile: both row bands
                src = bass.AP(
                    tensor=ps[:].tensor,
                    offset=ps[:].offset,
                    ap=[ps[:].ap[0], [PSROW, 2], [1, BAND]],
                )
                nc.scalar.copy(out=S[:, 2 * half : 2 * half + 2], in_=src)

            # gather1 -> mod-16 (8-elem chunks; static table)
            q16 = qp.tile([P, G1N, G1D], f16)
            nc.gpsimd.ap_gather(
                out_ap=q16[:],
                in_ap=S[:].rearrange("p a b -> p (a b)").rearrange(
                    "p (n d) -> p n d", d=G1D
                ),
                idxs_ap=g1_s[:],
                channels=P,
                num_elems=RPI * BAND // G1D,
                d=G1D,
                num_idxs=G1N,
            )

            qs = qsp.tile([P, RD * 24], f16)
            nc.vector.stream_shuffle(
                out=qs[:],
                in_=q16[:].rearrange("p a b -> p (a b)")[:, 0 : RD * 24],
                mask=SHUF,
            )

            # gather2 -> mod-8
            q8 = q8p.tile([P, G2N, G1D], f16)
            nc.gpsimd.ap_gather(
                out_ap=q8[:],
                in_ap=qs[:].rearrange("p (n d) -> p n d", d=G1D),
                idxs_ap=g2_s[:],
                channels=P,
                num_elems=RD * 24 // G1D,
                d=G1D,
                num_idxs=G2N,
            )

            # masked select: value for (p, rd, dxi) at q8[p, rd*16 + dxi + p%8].
            # Even dxi read q8 directly (4B-aligned); odd dxi read a +1-shifted
            # copy so their windows start even too -> everything runs 2x.
            q8a = q8[:].rearrange("p a b -> p (a b)")
            q8b = q8p.tile([P, RD * 16], f16, name="q8b", tag="q8b")
            shift_src = bass.AP(
                tensor=q8a.tensor, offset=q8a.offset + 1,
                ap=[q8a.ap[0], [1, RD * 16]],
            )
            nc.vector.tensor_copy(out=q8b[:], in_=shift_src)

            in_e = bass.AP(
                tensor=q8a.tensor, offset=q8a.offset,
                ap=[q8a.ap[0], [16, RD], [2, 5], [1, 8]],
            )
            q8ba = q8b[:]
            in_o = bass.AP(
                tensor=q8ba.tensor, offset=q8ba.offset,
                ap=[q8ba.ap[0], [16, RD], [2, 4], [1, 8]],
            )
            pe_ = pp.tile([P, RD, 5, 8], f16, name="pe", tag="pe")
            po_ = pp.tile([P, RD, 4, 8], f16, name="po", tag="po")
            nc.vector.tensor_mul(
                pe_[:], in_e,
                me_s[:].rearrange("p (a b c) -> p a b c", b=5, c=8),
            )
            nc.vector.tensor_mul(
                po_[:], in_o,
                mo_s[:].rearrange("p (a b c) -> p a b c", b=4, c=8),
            )

            t1e = op.tile([P, RD, 5, 4], f16, name="t1e", tag="t1e")
            t1o = op.tile([P, RD, 4, 4], f16, name="t1o", tag="t1o")
            nc.vector.tensor_add(t1e[:], pe_[:, :, :, 0:4], pe_[:, :, :, 4:8])
            nc.vector.tensor_add(t1o[:], po_[:, :, :, 0:4], po_[:, :, :, 4:8])
            t2e = op.tile([P, RD, 5, 2], f16, name="t2e", tag="t2e")
            t2o = op.tile([P, RD, 4, 2], f16, name="t2o", tag="t2o")
            nc.vector.tensor_add(t2e[:], t1e[:, :, :, 0:2], t1e[:, :, :, 2:4])
            nc.vector.tensor_add(t2o[:], t1o[:, :, :, 0:2], t1o[:, :, :, 2:4])

            O = op.tile([P, RD * ND], f16, name="O", tag="O")
            Oa = O[:]
            out_e = bass.AP(
                tensor=Oa.tensor, offset=Oa.offset,
                ap=[Oa.ap[0], [ND, RD], [2, 5]],
            )
            out_o = bass.AP(
                tensor=Oa.tensor, offset=Oa.offset + 1,
                ap=[Oa.ap[0], [ND, RD], [2, 4]],
            )
            nc.vector.tensor_add(out_e, t2e[:, :, :, 0], t2e[:, :, :, 1])
            nc.vector.tensor_add(out_o, t2o[:, :, :, 0], t2o[:, :, :, 1])

            # 4 transposes accumulate into one psum tile; identity = inverse
            # shuffle permutation so columns land at true pixel positions.
            pt = ptp.tile([P, RPI, P], f16, tag="pt")
            for row in range(RPI):
                nc.tensor.transpose(
                    out=pt[0:NCH, row, :],
                    in_=O[:, row * NCH : (row + 1) * NCH],
                    identity=perm_s[:],
                )
            Ost = stg.tile([NCH, RPI, W], f32)
            nc.scalar.copy(out=Ost[:], in_=pt[0:NCH, :, :])
            nc.sync.dma_start(out=out_t[:, h : h + RPI, :], in_=Ost[:])

    nc.finalize()
    return nc


def _get_nc():
    with _lock:
        if "nc" not in _cache:
            _cache["nc"] = _build_nc()
        return _cache["nc"]


def _in_maps(in1: np.ndarray, in2: np.ndarray):
    g1, g2, me, mo, perm = _host_tables()
    in1 = np.ascontiguousarray(in1, dtype=np.float32)
    in2 = np.ascontiguousarray(in2, dtype=np.float32)
    return [
        {
            "in1": in1[b],
            "in2": in2[b],
            "g1t": g1,
            "g2t": g2,
            "met": me,
            "mot": mo,
            "permt": perm,
        }
        for b in range(B)
    ]


def kernel(in1: np.ndarray, in2: np.ndarray) -> np.ndarray:
    from concourse.bass_utils import run_bass_kernel_spmd

    nc = _get_nc()
    in_maps = _in_maps(in1, in2)
    res = run_bass_kernel_spmd(nc, in_maps, core_ids=list(range(B)))
    out = np.stack([res.results[b]["out"] for b in range(B)], axis=0)
    return out
